# revision 10
# baseline (speedup 1.0000x reference)
"""Trainium2 Bass kernel for nn_AE_gnnrnn (embedding -> BiLSTM encoder ->
GCN -> per-step bidirectional LSTM decoder -> vocab logits).

Strategy (8 NeuronCores, data-parallel over the N=512 graph nodes, 64/core):
  - The output logits [512, 64, 4000] f32 (524 MB) dominate: the kernel is
    HBM-write bound (~65.5 MB/core). Decoder steps l>=1 use a fresh zero
    LSTM state, so 63/64 of the output depends only on x — phase A computes
    and streams those logits immediately.
  - Phase B (embedding gather -> BiLSTM scan over L=64 -> projections ->
    AllGather of the [512, 52] node states -> dense-normalized-adjacency
    GCN -> decoder step 0 -> l=0 logits) is interleaved under phase A.
  - GCN aggregation uses a host-precomputed symmetric-normalized adjacency
    (with self loops) as dense matmuls; layer-2 aggregation only computes
    this core's 64 destination columns.
  - Engine APs must start at 32-aligned partitions, so all gate blocks are
    padded to bases {0, 32, 64, 96} with zeros in the gaps.
Per-core output is [4096, 4000] with rows in l-major order (r = l*64 + n);
the host transposes back when unsharding.
"""
import sys

if '/opt/trn_rl_repo' not in sys.path:
    sys.path.insert(0, '/opt/trn_rl_repo')

import numpy as np

V, DIN, H, N, L, E = 4000, 64, 13, 512, 64, 16384
NC = 8
NLOC = N // NC          # 64 nodes per core
COLS = NLOC * L         # 4096 (n,l) rows per core
VCH = 500               # logits free-dim chunk (8 chunks of 500 = 4000)
# reference gate order in the 4H=52 vector: [ii, ff, gg, oo]
# padded partition base for each gate: ii@0, ff@32, oo@64, gg@96
GSRC = (0, 1, 3, 2)     # source 13-block for padded slots [0, 32, 64, 96]


def _f32(a):
    return np.ascontiguousarray(np.asarray(a, dtype=np.float32))


def _pad_gates_cols(M52):
    """[k, 52] -> [k, 128] with gate blocks at bases 0/32/64/96."""
    M52 = np.atleast_2d(M52)
    out = np.zeros((M52.shape[0], 128), np.float32)
    for j, blk in enumerate(GSRC):
        out[:, 32 * j:32 * j + 13] = M52[:, 13 * blk:13 * blk + 13]
    return out


def _wrap_idx16(flat_idx):
    n = len(flat_idx)
    blk = np.asarray(flat_idx, dtype=np.int16).reshape(n // 16, 16).T
    return np.ascontiguousarray(np.tile(blk, (8, 1)))  # [128, n/16]


def _make_host_inputs(x, edge_index, p):
    x = np.asarray(x)
    ei = np.asarray(edge_index)
    S = {}

    emb = _f32(p['emb'])
    embp = np.zeros((4096, DIN), np.float32)
    embp[:4001] = emb
    S['emb'] = embp
    S['ones'] = np.ones((1, COLS), np.float32)

    # encoder: lhsT [65, 128] = [Wih.T | bias row], gate-padded columns
    for sfx in ('f', 'b'):
        Wih = _f32(p[f'enc_Wih_{sfx}'])                    # [52, 64]
        b = _f32(p[f'enc_bih_{sfx}']) + _f32(p[f'enc_bhh_{sfx}'])
        aug = np.zeros((65, 128), np.float32)
        aug[:64] = _pad_gates_cols(Wih.T)
        aug[64] = _pad_gates_cols(b[None, :])[0]
        S[f'wstk_{sfx}'] = aug
        Whh = _f32(p[f'enc_Whh_{sfx}'])                    # [52, 13]
        S[f'whh_{sfx}'] = _pad_gates_cols(Whh.T)           # [13, 128]

    # projections: lhsT [128, 26], input rows hf@0:13, hb@32:45, ones@64
    for i, (w, b) in enumerate((('proj1_W', 'proj1_b'), ('proj2_W', 'proj2_b'))):
        aug = np.zeros((128, 26), np.float32)
        WT = _f32(p[w]).T                                  # [26, 26]
        aug[0:13] = WT[0:13]
        aug[32:45] = WT[13:26]
        aug[64] = _f32(p[b])
        S[f'proj{i + 1}'] = aug

    # GCN adjacency (normalized, with self loops)
    src = np.concatenate([ei[0], np.arange(N)]).astype(np.int64)
    dst = np.concatenate([ei[1], np.arange(N)]).astype(np.int64)
    deg = np.zeros(N, np.float64)
    np.add.at(deg, dst, 1.0)
    dinv = np.where(deg > 0, 1.0 / np.sqrt(deg), 0.0)
    A = np.zeros((N, N), np.float64)
    np.add.at(A, (dst, src), dinv[src] * dinv[dst])
    adjT = np.ascontiguousarray(A.T.astype(np.float32))    # adjT[s, d]
    S['adjT'] = adjT

    for g in ('gh', 'gc'):
        S[f'{g}_W1'] = _f32(p[f'{g}_W1'])
        S[f'{g}_W2'] = _f32(p[f'{g}_W2'])
        Wfc = _f32(p[f'{g}_Wfc'])                          # [32, 26]
        Wp = np.zeros((32, 46), np.float32)
        Wp[:, 0:13] = Wfc[:, 0:13]
        Wp[:, 32:45] = Wfc[:, 13:26]
        S[f'{g}_Wfc'] = Wp
        S[f'{g}_b1'] = _f32(p[f'{g}_b1']).reshape(16, 1)
        S[f'{g}_b2'] = _f32(p[f'{g}_b2']).reshape(32, 1)
        bfc = _f32(p[f'{g}_bfc'])
        bp = np.zeros((46, 1), np.float32)
        bp[0:13, 0] = bfc[0:13]
        bp[32:45, 0] = bfc[13:26]
        S[f'{g}_bfc'] = bp

    # decoder step 0: lhsT [64, 128]; rhs rows h0@0:13, ones@32
    for sfx in ('f', 'b'):
        Whh = _f32(p[f'dec_Whh_{sfx}'])                    # [52, 13]
        Wih0 = _f32(p[f'dec_Wih_{sfx}'])[:, 0]
        b = _f32(p[f'dec_bih_{sfx}']) + _f32(p[f'dec_bhh_{sfx}'])
        aug = np.zeros((64, 128), np.float32)
        aug[0:13] = _pad_gates_cols(Whh.T)
        aug[32] = _pad_gates_cols((b - Wih0)[None, :])[0]
        S[f'dec0_{sfx}'] = aug

    # decoder l>=1: lhsT [2, 128]; rows ii_fb@0:26, oo_f@32, oo_b@64, gg_fb@96
    Wf = _f32(p['dec_Wih_f'])[:, 0]
    Wb = _f32(p['dec_Wih_b'])[:, 0]
    bf = _f32(p['dec_bih_f']) + _f32(p['dec_bhh_f'])
    bb = _f32(p['dec_bih_b']) + _f32(p['dec_bhh_b'])
    pre = np.zeros((2, 128), np.float32)
    for row, vf, vb in ((0, Wf, Wb), (1, bf, bb)):
        pre[row, 0:13] = vf[0:13]      # ii_f
        pre[row, 13:26] = vb[0:13]     # ii_b
        pre[row, 32:45] = vf[39:52]    # oo_f
        pre[row, 45:58] = vb[39:52]    # oo_b
        pre[row, 64:77] = vf[26:39]    # gg_f
        pre[row, 77:90] = vb[26:39]    # gg_b
    S['dec_pre'] = pre

    # pack matrix: dec0_pad rows {0:13 -> 0:13, 32:45 -> 13:26, 96 -> 26}
    P27 = np.zeros((128, 27), np.float32)
    for k in range(13):
        P27[k, k] = 1.0
        P27[32 + k, 13 + k] = 1.0
    P27[96, 26] = 1.0
    S['packP27'] = P27

    Waug = np.zeros((27, V), np.float32)
    Waug[:26] = _f32(p['out_W']).T
    Waug[26] = _f32(p['out_b'])
    S['w_aug'] = Waug
    S['ident'] = np.eye(128, dtype=np.float32)

    per_core = []
    for c in range(NC):
        xc = x[c * NLOC:(c + 1) * NLOC]
        d = {}
        flat = np.ascontiguousarray(xc.T).reshape(-1)  # l-major: j = l*64+n
        d['xidx'] = _wrap_idx16(flat)
        vals = np.empty((NLOC, L), np.float32)
        vals[:, 0] = -1.0
        vals[:, 1:] = xc[:, :-1].astype(np.float32)
        v2 = np.empty((2, COLS), np.float32)
        v2[0] = np.ascontiguousarray(vals.T).reshape(-1)
        v2[1] = 1.0
        d['vals2'] = v2
        d['adjT_loc'] = np.ascontiguousarray(adjT[:, c * NLOC:(c + 1) * NLOC])
        per_core.append(d)
    return S, per_core


# ---------------------------------------------------------------------------
# device program
# ---------------------------------------------------------------------------
_BUILT = {}
import os as _os
FLAGS = {
    'gather': _os.environ.get('K_GATHER', '1') == '1',
    'collective': _os.environ.get('K_COLLECTIVE', '1') == '1',
    'scan': _os.environ.get('K_SCAN', '1') == '1',
    'phaseA': _os.environ.get('K_PHASEA', '1') == '1',
    'tail': _os.environ.get('K_TAIL', '1') == '1',
}

INPUT_SPECS = [
    ('emb', [4096, DIN], 'f32'), ('xidx', [128, 256], 'i16'),
    ('vals2', [2, COLS], 'f32'), ('ident', [128, 128], 'f32'),
    ('ones', [1, COLS], 'f32'),
    ('wstk_f', [65, 128], 'f32'), ('wstk_b', [65, 128], 'f32'),
    ('whh_f', [13, 128], 'f32'), ('whh_b', [13, 128], 'f32'),
    ('proj1', [128, 26], 'f32'), ('proj2', [128, 26], 'f32'),
    ('adjT', [512, 512], 'f32'), ('adjT_loc', [512, NLOC], 'f32'),
    ('gh_W1', [26, 16], 'f32'), ('gh_W2', [16, 32], 'f32'),
    ('gh_Wfc', [32, 46], 'f32'), ('gh_b1', [16, 1], 'f32'),
    ('gh_b2', [32, 1], 'f32'), ('gh_bfc', [46, 1], 'f32'),
    ('gc_W1', [26, 16], 'f32'), ('gc_W2', [16, 32], 'f32'),
    ('gc_Wfc', [32, 46], 'f32'), ('gc_b1', [16, 1], 'f32'),
    ('gc_b2', [32, 1], 'f32'), ('gc_bfc', [46, 1], 'f32'),
    ('dec0_f', [64, 128], 'f32'), ('dec0_b', [64, 128], 'f32'),
    ('dec_pre', [2, 128], 'f32'), ('packP27', [128, 27], 'f32'),
    ('w_aug', [27, V], 'f32'),
]


def _build():
    if 'nc' in _BUILT:
        return _BUILT['nc']
    from concourse import bass, bacc, tile, mybir

    dt = mybir.dt.float32
    AF = mybir.ActivationFunctionType
    nc = bacc.Bacc("TRN2", target_bir_lowering=False, debug=False,
                   num_devices=NC)

    dram = {}
    for name, shape, kind in INPUT_SPECS:
        dtt = mybir.dt.int16 if kind == 'i16' else dt
        dram[name] = nc.dram_tensor(name, shape, dtt, kind="ExternalInput")
    dram['out'] = nc.dram_tensor('out', [COLS, V], dt, kind="ExternalOutput")
    dram['cc_in'] = nc.dram_tensor('cc_in', [NLOC, 64], dt)
    dram['cc_out'] = nc.dram_tensor('cc_out', [N, 64], dt, addr_space="Shared")

    with tile.TileContext(nc) as tc:
        _body(nc, tc, bass, mybir, AF, dram)
    nc.compile()
    _BUILT['nc'] = nc
    return nc


def _body(nc, tc, bass, mybir, AF, dram):
    dt = mybir.dt.float32
    from contextlib import ExitStack

    with ExitStack() as ctx:
        const = ctx.enter_context(tc.tile_pool(name="const", bufs=1))
        psA = ctx.enter_context(
            tc.tile_pool(name="psA", bufs=2, space=bass.MemorySpace.PSUM))
        psB = ctx.enter_context(
            tc.tile_pool(name="psB", bufs=2, space=bass.MemorySpace.PSUM))
        psM = ctx.enter_context(
            tc.tile_pool(name="psM", bufs=2, space=bass.MemorySpace.PSUM))
        work = ctx.enter_context(tc.tile_pool(name="work", bufs=2))

        # ---------------- const loads ----------------
        cw = {}
        for name, shape, kind in INPUT_SPECS:
            if name in ('emb', 'ones'):
                continue
            dtt = mybir.dt.int16 if kind == 'i16' else dt
            if name == 'adjT':
                t = const.tile([128, 4 * 512], dt, tag=name, name='c_adjT')
                for k in range(4):
                    nc.sync.dma_start(out=t[:, 512 * k:512 * (k + 1)],
                                      in_=dram[name][128 * k:128 * (k + 1), :])
            elif name == 'adjT_loc':
                t = const.tile([128, 4 * NLOC], dt, tag=name, name='c_adjL')
                for k in range(4):
                    nc.sync.dma_start(out=t[:, NLOC * k:NLOC * (k + 1)],
                                      in_=dram[name][128 * k:128 * (k + 1), :])
            else:
                t = const.tile(shape, dtt, tag=name, name='c_' + name)
                nc.sync.dma_start(out=t[:], in_=dram[name][:])
            cw[name] = t

        ident = cw['ident']
        vals2 = cw['vals2']
        waug = cw['w_aug']

        decT = const.tile([27, COLS], dt, tag="decT", name="decT")
        # ones row for the logits bias (DMA: engines can't write partition 26)
        nc.sync.dma_start(out=decT[26:27, :], in_=dram['ones'][:])

        XW = {}
        XW[0] = const.tile([128, COLS], dt, tag="XW_f", name="XW_f")
        XW[1] = const.tile([128, COLS], dt, tag="XW_b", name="XW_b")

        # scan state: per-direction h @ base 0 (matmul rhs), c @ base 32
        # (tensor_tensor inputs must share a partition base with the ff gate)
        hst, cst = {}, {}
        for di in (0, 1):
            hst[di] = const.tile([13, NLOC], dt, tag=f"h{di}", name=f"h{di}")
            cst[di] = const.tile([45, NLOC], dt, tag=f"c{di}", name=f"c{di}")
            nc.vector.memset(hst[di][:], 0.0)
            nc.vector.memset(cst[di][32:45, :], 0.0)

        # ---------------- phase B head: gather + XW ----------------
        with tc.tile_pool(name="setup", bufs=1) as setup:
            embG = setup.tile([128, 32, DIN], dt, tag="embG", name="embG")
            if FLAGS['gather']:
                # split: one SWDGE ring holds 128 in-flight descriptors
                GCH = int(_os.environ.get('K_GCH', '512'))
                nch = GCH // 128
                for k in range(COLS // GCH):
                    nc.gpsimd.dma_gather(
                        embG[:, nch * k:nch * (k + 1), :], dram['emb'][:],
                        cw['xidx'][:, (GCH // 16) * k:(GCH // 16) * (k + 1)],
                        num_idxs=GCH, num_idxs_reg=GCH, elem_size=DIN)
            else:
                nc.vector.memset(embG[:], 0.01)
            embT = setup.tile([65, COLS], dt, tag="embT", name="embT")
            nc.vector.memset(embT[64:65, :], 1.0)
            for k in range(32):
                tp = psM.tile([128, 512], dt, tag="m", name=f"tpE{k}")
                nc.tensor.transpose(tp[0:DIN, 0:128], embG[:, k, :], ident[:])
                nc.scalar.copy(embT[0:DIN, 128 * k:128 * (k + 1)],
                               tp[0:DIN, 0:128])
            for ch in range(8):
                s = slice(512 * ch, 512 * (ch + 1))
                for di, wn in ((0, 'wstk_f'), (1, 'wstk_b')):
                    pf = psM.tile([128, 512], dt, tag="m", name=f"pXW{di}_{ch}")
                    nc.tensor.matmul(pf[:], cw[wn][:], embT[:, s])
                    nc.vector.tensor_copy(XW[di][:, s], pf[:])

        # ---------------- interleaved scan + phase A ----------------
        lt_pool = ctx.enter_context(tc.tile_pool(name="lt", bufs=2))
        enc = ctx.enter_context(tc.tile_pool(name="enc", bufs=2))
        act = ctx.enter_context(tc.tile_pool(name="act", bufs=2))

        ltiles = {}

        def emit_chunk(c):
            """decoder l>=1 cell for cols [512c, 512c+512) -> decT."""
            s = slice(512 * c, 512 * (c + 1))
            pre = psM.tile([128, 512], dt, tag="m", name=f"pre{c}")
            nc.tensor.matmul(pre[:], cw['dec_pre'][:], vals2[:, s])
            sgA = act.tile([58, 512], dt, tag="sgA", name=f"sgA{c}")
            tgA = act.tile([26, 512], dt, tag="tgA", name=f"tgA{c}")
            nc.scalar.activation(sgA[:], pre[0:58, :], AF.Sigmoid)
            nc.scalar.activation(tgA[:], pre[64:90, :], AF.Tanh)
            cD = act.tile([26, 512], dt, tag="cD", name=f"cD{c}")
            nc.vector.tensor_mul(cD[:], sgA[0:26, :], tgA[:])
            # thD lives at base 32 so the hD mul's inputs share a base
            thD = act.tile([58, 512], dt, tag="thD", name=f"thD{c}")
            nc.scalar.activation(thD[32:58, :], cD[:], AF.Tanh)
            nc.vector.tensor_mul(decT[0:26, s], sgA[32:58, :], thD[32:58, :])

        def emit_tile_half(t, half):
            """4 of 8 logits matmuls + copies for row-tile t; DMA at half 1."""
            lhs = decT[:, 128 * t:128 * (t + 1)]
            if half == 0:
                ltiles[t] = lt_pool.tile([128, V], dt, tag="ltile",
                                         name=f"ltile{t}")
            sb = ltiles[t]
            for j in range(4 * half, 4 * half + 4):
                s = slice(VCH * j, VCH * (j + 1))
                ps = psA.tile([128, VCH], dt, tag="ps", name=f"lg{t}_{j}")
                nc.tensor.matmul(ps[:], lhs, waug[:, s])
                if j < 6:
                    nc.vector.tensor_copy(sb[:, s], ps[:])
                else:
                    nc.scalar.copy(sb[:, s], ps[:])
            if half == 1:
                if t == 0:
                    nc.sync.dma_start(out=dram['out'][64:128, :],
                                      in_=sb[64:128, :])
                else:
                    nc.sync.dma_start(
                        out=dram['out'][128 * t:128 * (t + 1), :], in_=sb[:])
                del ltiles[t]

        def emit_enc_step(t):
            for di, wn in ((0, 'whh_f'), (1, 'whh_b')):
                lcol = t if di == 0 else 63 - t
                g = psB.tile([128, NLOC], dt, tag=f"g{di}", name=f"g{di}_{t}")
                nc.tensor.matmul(g[:], cw[wn][:], hst[di][:],
                                 start=True, stop=False)
                nc.tensor.matmul(g[:], ident[:],
                                 XW[di][:, 64 * lcol:64 * (lcol + 1)],
                                 start=False, stop=True)
                sg = enc.tile([77, NLOC], dt, tag=f"sg{di}", name=f"sg{di}_{t}")
                nc.scalar.activation(sg[:], g[0:77, :], AF.Sigmoid)
                tg = enc.tile([13, NLOC], dt, tag=f"tg{di}", name=f"tg{di}_{t}")
                nc.scalar.activation(tg[:], g[96:109, :], AF.Tanh)
                t1 = enc.tile([45, NLOC], dt, tag=f"t1{di}", name=f"t1{di}_{t}")
                nc.vector.tensor_mul(t1[32:45, :], sg[0:13, :], tg[:])
                cs = cst[di]
                nc.vector.tensor_mul(cs[32:45, :], sg[32:45, :], cs[32:45, :])
                nc.vector.tensor_add(cs[32:45, :], cs[32:45, :], t1[32:45, :])
                th = enc.tile([77, NLOC], dt, tag=f"th{di}", name=f"th{di}_{t}")
                nc.scalar.activation(th[64:77, :], cs[32:45, :], AF.Tanh)
                nc.vector.tensor_mul(hst[di][:], sg[64:77, :], th[64:77, :])

        for t in range(L):
            if FLAGS['scan']:
                emit_enc_step(t)
            if FLAGS['phaseA']:
                if t % 8 == 0:
                    emit_chunk(t // 8)
                emit_tile_half(t // 2, t % 2)

        # ---------------- phase B tail ----------------
        if not FLAGS['tail']:
            dummy = const.tile([1, 16], dt, tag="dummy", name="dummy")
            nc.vector.memset(dummy[:], 0.0)
            nc.sync.dma_start(out=dram['out'][0:1, 0:16], in_=dummy[:])
            return
        # assemble padded [hf@0:13, hb@32:45, ones@64] rhs tiles
        hboth = const.tile([128, NLOC], dt, tag="hboth", name="hboth")
        cboth = const.tile([128, NLOC], dt, tag="cboth", name="cboth")
        nc.vector.memset(hboth[:], 0.0)
        nc.vector.tensor_copy(hboth[0:13, :], hst[0][:])
        nc.vector.tensor_copy(hboth[32:45, :], hst[1][:])
        nc.vector.memset(hboth[64:65, :], 1.0)
        nc.vector.memset(cboth[:], 0.0)
        nc.vector.tensor_copy(cboth[0:13, :], cst[0][32:45, :])
        nc.vector.tensor_copy(cboth[32:45, :], cst[1][32:45, :])
        nc.vector.memset(cboth[64:65, :], 1.0)
        pst = psM.tile([128, 512], dt, tag="m", name="pst")
        nc.tensor.matmul(pst[0:26, 0:NLOC], cw['proj1'][:], hboth[:],
                         skip_group_check=True)
        nc.tensor.matmul(pst[32:58, 0:NLOC], cw['proj2'][:], cboth[:],
                         skip_group_check=True)
        state = const.tile([64, NLOC], dt, tag="state", name="state")
        nc.vector.memset(state[:], 0.0)
        nc.vector.tensor_copy(state[0:26, :], pst[0:26, 0:NLOC])
        nc.vector.tensor_copy(state[32:58, :], pst[32:58, 0:NLOC])
        pstN = psM.tile([128, 512], dt, tag="m", name="pstN")
        nc.tensor.transpose(pstN[0:NLOC, 0:64], state[:], ident[0:64, 0:64])
        stateN = const.tile([NLOC, 64], dt, tag="stateN", name="stateN")
        nc.vector.tensor_copy(stateN[:], pstN[0:NLOC, 0:64])
        nc.sync.dma_start(out=dram['cc_in'][:], in_=stateN[:])
        if FLAGS['collective']:
            nc.gpsimd.collective_compute(
                "AllGather", mybir.AluOpType.bypass,
                ins=[dram['cc_in'][:]], outs=[dram['cc_out'][:]],
                replica_groups=[list(range(NC))])
        else:
            for _r in range(NC):
                nc.sync.dma_start(out=dram['cc_out'][_r * NLOC:(_r + 1) * NLOC, :],
                                  in_=dram['cc_in'][:])
        # read back + transpose to [26, 512] x2 (h rows 0:26, c rows 32:58)
        XhT = const.tile([26, N], dt, tag="XhT", name="XhT")
        XcT = const.tile([26, N], dt, tag="XcT", name="XcT")
        for k in range(4):
            xn = work.tile([128, 64], dt, tag="xn", name=f"xn{k}")
            nc.sync.dma_start(out=xn[:],
                              in_=dram['cc_out'][128 * k:128 * (k + 1), :])
            tp = psM.tile([128, 512], dt, tag="m", name=f"tpX{k}")
            nc.tensor.transpose(tp[0:64, 0:128], xn[:], ident[:])
            nc.vector.tensor_copy(XhT[:, 128 * k:128 * (k + 1)],
                                  tp[0:26, 0:128])
            nc.vector.tensor_copy(XcT[:, 128 * k:128 * (k + 1)],
                                  tp[32:58, 0:128])

        # GCN x2 (gh on XhT, gc on XcT) -> padded [46, 64] local outputs
        outT = {}
        for g, XT in (('gh', XhT), ('gc', XcT)):
            m1 = work.tile([128, 64], dt, tag="m1", name=f"m1{g}")
            for k in range(4):
                pm = psM.tile([128, 512], dt, tag="m", name=f"pm1{g}{k}")
                nc.tensor.matmul(pm[0:128, 0:16],
                                 XT[:, 128 * k:128 * (k + 1)], cw[f'{g}_W1'][:])
                nc.vector.tensor_copy(m1[:, 16 * k:16 * (k + 1)],
                                      pm[0:128, 0:16])
            agg1 = psM.tile([128, 512], dt, tag="m", name=f"agg1{g}")
            for k in range(4):
                nc.tensor.matmul(agg1[0:16, :], m1[:, 16 * k:16 * (k + 1)],
                                 cw['adjT'][:, 512 * k:512 * (k + 1)],
                                 start=(k == 0), stop=(k == 3))
            xn1 = work.tile([16, N], dt, tag="xn1", name=f"xn1{g}")
            nc.scalar.activation(xn1[:], agg1[0:16, :], AF.Lrelu,
                                 bias=cw[f'{g}_b1'][:], alpha=0.01)
            m2 = work.tile([128, 128], dt, tag="m2", name=f"m2{g}")
            for k in range(4):
                pm = psM.tile([128, 512], dt, tag="m", name=f"pm2{g}{k}")
                nc.tensor.matmul(pm[0:128, 0:32],
                                 xn1[:, 128 * k:128 * (k + 1)], cw[f'{g}_W2'][:])
                nc.vector.tensor_copy(m2[:, 32 * k:32 * (k + 1)],
                                      pm[0:128, 0:32])
            agg2 = psM.tile([128, 512], dt, tag="m", name=f"agg2{g}")
            for k in range(4):
                nc.tensor.matmul(agg2[0:32, 0:NLOC], m2[:, 32 * k:32 * (k + 1)],
                                 cw['adjT_loc'][:, NLOC * k:NLOC * (k + 1)],
                                 start=(k == 0), stop=(k == 3))
            xn2 = work.tile([32, NLOC], dt, tag="xn2", name=f"xn2{g}")
            nc.scalar.activation(xn2[:], agg2[0:32, 0:NLOC], AF.Lrelu,
                                 bias=cw[f'{g}_b2'][:], alpha=0.01)
            pf = psM.tile([128, 512], dt, tag="m", name=f"pf{g}")
            nc.tensor.matmul(pf[0:46, 0:NLOC], cw[f'{g}_Wfc'][:], xn2[:])
            oT = work.tile([46, NLOC], dt, tag=f"oT{g}", name=f"oT{g}")
            nc.scalar.activation(oT[:], pf[0:46, 0:NLOC], AF.Identity,
                                 bias=cw[f'{g}_bfc'][:])
            outT[g] = oT
        shT, scT = outT['gh'], outT['gc']

        # decoder step 0 -> dec0_pad [hF@0:13, hB@32:45, ones@96]
        dec0p = const.tile([128, NLOC], dt, tag="dec0p", name="dec0p")
        nc.vector.memset(dec0p[:], 0.0)
        nc.vector.memset(dec0p[96:97, :], 1.0)
        for di, wn in ((0, 'dec0_f'), (1, 'dec0_b')):
            h0 = work.tile([64, NLOC], dt, tag=f"h0{di}", name=f"h0{di}")
            nc.vector.memset(h0[:], 0.0)
            nc.vector.tensor_copy(h0[0:13, :], shT[32 * di:32 * di + 13, :])
            nc.vector.memset(h0[32:33, :], 1.0)
            g0 = psM.tile([128, 512], dt, tag="m", name=f"g0{di}")
            nc.tensor.matmul(g0[:, 0:NLOC], cw[wn][:], h0[:])
            sg = work.tile([77, NLOC], dt, tag=f"d0sg{di}", name=f"d0sg{di}")
            nc.scalar.activation(sg[:], g0[0:77, 0:NLOC], AF.Sigmoid)
            tg = work.tile([13, NLOC], dt, tag=f"d0tg{di}", name=f"d0tg{di}")
            nc.scalar.activation(tg[:], g0[96:109, 0:NLOC], AF.Tanh)
            sc0 = work.tile([45, NLOC], dt, tag=f"d0sc{di}", name=f"d0sc{di}")
            nc.vector.tensor_copy(sc0[32:45, :], scT[32 * di:32 * di + 13, :])
            c0 = work.tile([45, NLOC], dt, tag=f"d0c{di}", name=f"d0c{di}")
            nc.vector.tensor_mul(c0[32:45, :], sg[0:13, :], tg[:])
            ct = work.tile([45, NLOC], dt, tag=f"d0ct{di}", name=f"d0ct{di}")
            nc.vector.tensor_mul(ct[32:45, :], sg[32:45, :], sc0[32:45, :])
            nc.vector.tensor_add(c0[32:45, :], c0[32:45, :], ct[32:45, :])
            th = work.tile([77, NLOC], dt, tag=f"d0th{di}", name=f"d0th{di}")
            nc.scalar.activation(th[64:77, :], c0[32:45, :], AF.Tanh)
            nc.vector.tensor_mul(dec0p[32 * di:32 * di + 13, :],
                                 sg[64:77, :], th[64:77, :])

        # pack to [27, 64] and emit l=0 logits -> out rows 0:64
        ppk = psM.tile([128, 512], dt, tag="m", name="ppk")
        nc.tensor.matmul(ppk[0:27, 0:NLOC], cw['packP27'][:], dec0p[:])
        dec27 = const.tile([27, NLOC], dt, tag="dec27", name="dec27")
        nc.vector.tensor_copy(dec27[:], ppk[0:27, 0:NLOC])
        l0 = const.tile([NLOC, V], dt, tag="l0", name="l0")
        for j in range(8):
            s = slice(VCH * j, VCH * (j + 1))
            ps = psA.tile([128, VCH], dt, tag="ps", name=f"l0g{j}")
            nc.tensor.matmul(ps[0:NLOC, :], dec27[:], waug[:, s])
            nc.vector.tensor_copy(l0[:, s], ps[0:NLOC, :])
        nc.sync.dma_start(out=dram['out'][0:NLOC, :], in_=l0[:])


# ---------------------------------------------------------------------------
# entry point
# ---------------------------------------------------------------------------
def kernel(x=None, edge_index=None, params=None, **kw):
    from concourse.bass_utils import run_bass_kernel_spmd
    S, P = _make_host_inputs(x, edge_index, params)
    nc = _build()

    per_core_keys = {'xidx', 'vals2', 'adjT_loc'}
    in_maps = []
    for c in range(NC):
        m = {}
        for name, _, _ in INPUT_SPECS:
            m[name] = P[c][name] if name in per_core_keys else S[name]
        in_maps.append(m)

    res = run_bass_kernel_spmd(nc, in_maps, core_ids=list(range(NC)), **kw)
    out = np.empty((N, L, V), np.float32)
    for c in range(NC):
        shard = res.results[c]['out'].reshape(L, NLOC, V)
        out[c * NLOC:(c + 1) * NLOC] = shard.transpose(1, 0, 2)
    if kw.get('trace'):
        kernel.last_exec_time_ns = res.exec_time_ns
    return out


# revision 12
# speedup vs baseline: 1.3973x; 1.3973x over previous
"""Trainium2 Bass kernel for nn_AE_gnnrnn (embedding -> BiLSTM encoder ->
GCN -> per-step bidirectional LSTM decoder -> vocab logits).

Strategy (8 NeuronCores, data-parallel over the N=512 graph nodes, 64/core):
  - The output logits [512, 64, 4000] f32 (524 MB) dominate: the kernel is
    HBM-write bound (~65.5 MB/core). Decoder steps l>=1 use a fresh zero
    LSTM state, so 63/64 of the output depends only on x — phase A computes
    and streams those logits immediately.
  - Phase B (embedding gather -> BiLSTM scan over L=64 -> projections ->
    AllGather of the [512, 52] node states -> dense-normalized-adjacency
    GCN -> decoder step 0 -> l=0 logits) is interleaved under phase A.
  - GCN aggregation uses a host-precomputed symmetric-normalized adjacency
    (with self loops) as dense matmuls; layer-2 aggregation only computes
    this core's 64 destination columns.
  - Engine APs must start at 32-aligned partitions, so all gate blocks are
    padded to bases {0, 32, 64, 96} with zeros in the gaps.
Per-core output is [4096, 4000] with rows in l-major order (r = l*64 + n);
the host transposes back when unsharding.
"""
import sys

if '/opt/trn_rl_repo' not in sys.path:
    sys.path.insert(0, '/opt/trn_rl_repo')

import numpy as np

V, DIN, H, N, L, E = 4000, 64, 13, 512, 64, 16384
NC = 8
NLOC = N // NC          # 64 nodes per core
COLS = NLOC * L         # 4096 (n,l) rows per core
VCH = 500               # logits free-dim chunk (8 chunks of 500 = 4000)
# reference gate order in the 4H=52 vector: [ii, ff, gg, oo]
# padded partition base for each gate: ii@0, ff@32, oo@64, gg@96
GSRC = (0, 1, 3, 2)     # source 13-block for padded slots [0, 32, 64, 96]


def _f32(a):
    return np.ascontiguousarray(np.asarray(a, dtype=np.float32))


def _pad_gates_cols(M52):
    """[k, 52] -> [k, 128] with gate blocks at bases 0/32/64/96."""
    M52 = np.atleast_2d(M52)
    out = np.zeros((M52.shape[0], 128), np.float32)
    for j, blk in enumerate(GSRC):
        out[:, 32 * j:32 * j + 13] = M52[:, 13 * blk:13 * blk + 13]
    return out


def _wrap_idx16(flat_idx):
    n = len(flat_idx)
    blk = np.asarray(flat_idx, dtype=np.int16).reshape(n // 16, 16).T
    return np.ascontiguousarray(np.tile(blk, (8, 1)))  # [128, n/16]


def _make_host_inputs(x, edge_index, p):
    x = np.asarray(x)
    ei = np.asarray(edge_index)
    S = {}

    emb = _f32(p['emb'])
    embp = np.zeros((4096, DIN), np.float32)
    embp[:4001] = emb
    S['emb'] = embp
    S['ones'] = np.ones((1, COLS), np.float32)

    # encoder: lhsT [65, 128] = [Wih.T | bias row], gate-padded columns
    for sfx in ('f', 'b'):
        Wih = _f32(p[f'enc_Wih_{sfx}'])                    # [52, 64]
        b = _f32(p[f'enc_bih_{sfx}']) + _f32(p[f'enc_bhh_{sfx}'])
        aug = np.zeros((65, 128), np.float32)
        aug[:64] = _pad_gates_cols(Wih.T)
        aug[64] = _pad_gates_cols(b[None, :])[0]
        S[f'wstk_{sfx}'] = aug
    whh_c = np.zeros((45, 128), np.float32)
    whh_c[0:13] = _pad_gates_cols(_f32(p['enc_Whh_f']).T)
    whh_c[32:45] = _pad_gates_cols(_f32(p['enc_Whh_b']).T)
    S['whh_c'] = whh_c

    # projections: lhsT [128, 26], input rows hf@0:13, hb@32:45, ones@64
    for i, (w, b) in enumerate((('proj1_W', 'proj1_b'), ('proj2_W', 'proj2_b'))):
        aug = np.zeros((128, 26), np.float32)
        WT = _f32(p[w]).T                                  # [26, 26]
        aug[0:13] = WT[0:13]
        aug[32:45] = WT[13:26]
        aug[64] = _f32(p[b])
        S[f'proj{i + 1}'] = aug

    # GCN adjacency (normalized, with self loops)
    src = np.concatenate([ei[0], np.arange(N)]).astype(np.int64)
    dst = np.concatenate([ei[1], np.arange(N)]).astype(np.int64)
    deg = np.zeros(N, np.float64)
    np.add.at(deg, dst, 1.0)
    dinv = np.where(deg > 0, 1.0 / np.sqrt(deg), 0.0)
    A = np.zeros((N, N), np.float64)
    np.add.at(A, (dst, src), dinv[src] * dinv[dst])
    adjT = np.ascontiguousarray(A.T.astype(np.float32))    # adjT[s, d]
    S['adjT'] = adjT

    for g in ('gh', 'gc'):
        S[f'{g}_W1'] = _f32(p[f'{g}_W1'])
        S[f'{g}_W2'] = _f32(p[f'{g}_W2'])
        Wfc = _f32(p[f'{g}_Wfc'])                          # [32, 26]
        Wp = np.zeros((32, 46), np.float32)
        Wp[:, 0:13] = Wfc[:, 0:13]
        Wp[:, 32:45] = Wfc[:, 13:26]
        S[f'{g}_Wfc'] = Wp
        S[f'{g}_b1'] = _f32(p[f'{g}_b1']).reshape(16, 1)
        S[f'{g}_b2'] = _f32(p[f'{g}_b2']).reshape(32, 1)
        bfc = _f32(p[f'{g}_bfc'])
        bp = np.zeros((46, 1), np.float32)
        bp[0:13, 0] = bfc[0:13]
        bp[32:45, 0] = bfc[13:26]
        S[f'{g}_bfc'] = bp

    # decoder step 0: lhsT [64, 128]; rhs rows h0@0:13, ones@32
    for sfx in ('f', 'b'):
        Whh = _f32(p[f'dec_Whh_{sfx}'])                    # [52, 13]
        Wih0 = _f32(p[f'dec_Wih_{sfx}'])[:, 0]
        b = _f32(p[f'dec_bih_{sfx}']) + _f32(p[f'dec_bhh_{sfx}'])
        aug = np.zeros((64, 128), np.float32)
        aug[0:13] = _pad_gates_cols(Whh.T)
        aug[32] = _pad_gates_cols((b - Wih0)[None, :])[0]
        S[f'dec0_{sfx}'] = aug

    # decoder l>=1: lhsT [2, 128]; rows ii_fb@0:26, oo_f@32, oo_b@64, gg_fb@96
    Wf = _f32(p['dec_Wih_f'])[:, 0]
    Wb = _f32(p['dec_Wih_b'])[:, 0]
    bf = _f32(p['dec_bih_f']) + _f32(p['dec_bhh_f'])
    bb = _f32(p['dec_bih_b']) + _f32(p['dec_bhh_b'])
    pre = np.zeros((2, 128), np.float32)
    for row, vf, vb in ((0, Wf, Wb), (1, bf, bb)):
        pre[row, 0:13] = vf[0:13]      # ii_f
        pre[row, 13:26] = vb[0:13]     # ii_b
        pre[row, 32:45] = vf[39:52]    # oo_f
        pre[row, 45:58] = vb[39:52]    # oo_b
        pre[row, 64:77] = vf[26:39]    # gg_f
        pre[row, 77:90] = vb[26:39]    # gg_b
    S['dec_pre'] = pre

    # pack matrix: dec0_pad rows {0:13 -> 0:13, 32:45 -> 13:26, 96 -> 26}
    P27 = np.zeros((128, 27), np.float32)
    for k in range(13):
        P27[k, k] = 1.0
        P27[32 + k, 13 + k] = 1.0
    P27[96, 26] = 1.0
    S['packP27'] = P27

    Waug = np.zeros((27, V), np.float32)
    Waug[:26] = _f32(p['out_W']).T
    Waug[26] = _f32(p['out_b'])
    S['w_aug'] = Waug
    S['ident'] = np.eye(128, dtype=np.float32)

    per_core = []
    for c in range(NC):
        xc = x[c * NLOC:(c + 1) * NLOC]
        d = {}
        flat = np.ascontiguousarray(xc.T).reshape(-1)  # l-major: j = l*64+n
        d['xidx'] = _wrap_idx16(flat)
        vals = np.empty((NLOC, L), np.float32)
        vals[:, 0] = -1.0
        vals[:, 1:] = xc[:, :-1].astype(np.float32)
        v2 = np.empty((2, COLS), np.float32)
        v2[0] = np.ascontiguousarray(vals.T).reshape(-1)
        v2[1] = 1.0
        d['vals2'] = v2
        d['adjT_loc'] = np.ascontiguousarray(adjT[:, c * NLOC:(c + 1) * NLOC])
        per_core.append(d)
    return S, per_core


# ---------------------------------------------------------------------------
# device program
# ---------------------------------------------------------------------------
_BUILT = {}
import os as _os
FLAGS = {
    'gather': _os.environ.get('K_GATHER', '1') == '1',
    'collective': _os.environ.get('K_COLLECTIVE', '1') == '1',
    'scan': _os.environ.get('K_SCAN', '1') == '1',
    'phaseA': _os.environ.get('K_PHASEA', '1') == '1',
    'tail': _os.environ.get('K_TAIL', '1') == '1',
}

INPUT_SPECS = [
    ('emb', [4096, DIN], 'f32'), ('xidx', [128, 256], 'i16'),
    ('vals2', [2, COLS], 'f32'), ('ident', [128, 128], 'f32'),
    ('ones', [1, COLS], 'bf16'),
    ('wstk_f', [65, 128], 'f32'), ('wstk_b', [65, 128], 'f32'),
    ('whh_c', [45, 128], 'f32'),
    ('proj1', [128, 26], 'f32'), ('proj2', [128, 26], 'f32'),
    ('adjT', [512, 512], 'f32'), ('adjT_loc', [512, NLOC], 'f32'),
    ('gh_W1', [26, 16], 'f32'), ('gh_W2', [16, 32], 'f32'),
    ('gh_Wfc', [32, 46], 'f32'), ('gh_b1', [16, 1], 'f32'),
    ('gh_b2', [32, 1], 'f32'), ('gh_bfc', [46, 1], 'f32'),
    ('gc_W1', [26, 16], 'f32'), ('gc_W2', [16, 32], 'f32'),
    ('gc_Wfc', [32, 46], 'f32'), ('gc_b1', [16, 1], 'f32'),
    ('gc_b2', [32, 1], 'f32'), ('gc_bfc', [46, 1], 'f32'),
    ('dec0_f', [64, 128], 'f32'), ('dec0_b', [64, 128], 'f32'),
    ('dec_pre', [2, 128], 'f32'), ('packP27', [128, 27], 'f32'),
    ('w_aug', [27, V], 'bf16'),
]


def _build():
    if 'nc' in _BUILT:
        return _BUILT['nc']
    from concourse import bass, bacc, tile, mybir

    dt = mybir.dt.float32
    AF = mybir.ActivationFunctionType
    nc = bacc.Bacc("TRN2", target_bir_lowering=False, debug=False,
                   num_devices=NC)

    dram = {}
    KIND2DT = {'i16': mybir.dt.int16, 'bf16': mybir.dt.bfloat16, 'f32': dt}
    for name, shape, kind in INPUT_SPECS:
        dram[name] = nc.dram_tensor(name, shape, KIND2DT[kind],
                                    kind="ExternalInput")
    dram['out'] = nc.dram_tensor('out', [COLS, V], dt, kind="ExternalOutput")
    dram['cc_in'] = nc.dram_tensor('cc_in', [NLOC, 64], dt)
    dram['cc_out'] = nc.dram_tensor('cc_out', [N, 64], dt, addr_space="Shared")

    with tile.TileContext(nc) as tc:
        _body(nc, tc, bass, mybir, AF, dram)
    nc.compile()
    _BUILT['nc'] = nc
    return nc


def _body(nc, tc, bass, mybir, AF, dram):
    dt = mybir.dt.float32
    from contextlib import ExitStack

    with ExitStack() as ctx:
        const = ctx.enter_context(tc.tile_pool(name="const", bufs=1))
        psA = ctx.enter_context(
            tc.tile_pool(name="psA", bufs=2, space=bass.MemorySpace.PSUM))
        psB = ctx.enter_context(
            tc.tile_pool(name="psB", bufs=2, space=bass.MemorySpace.PSUM))
        psM = ctx.enter_context(
            tc.tile_pool(name="psM", bufs=2, space=bass.MemorySpace.PSUM))
        work = ctx.enter_context(tc.tile_pool(name="work", bufs=2))

        # ---------------- const loads ----------------
        cw = {}
        KIND2DT = {'i16': mybir.dt.int16, 'bf16': mybir.dt.bfloat16, 'f32': dt}
        for name, shape, kind in INPUT_SPECS:
            if name in ('emb', 'ones'):
                continue
            dtt = KIND2DT[kind]
            if name == 'adjT':
                t = const.tile([128, 4 * 512], dt, tag=name, name='c_adjT')
                for k in range(4):
                    nc.sync.dma_start(out=t[:, 512 * k:512 * (k + 1)],
                                      in_=dram[name][128 * k:128 * (k + 1), :])
            elif name == 'adjT_loc':
                t = const.tile([128, 4 * NLOC], dt, tag=name, name='c_adjL')
                for k in range(4):
                    nc.sync.dma_start(out=t[:, NLOC * k:NLOC * (k + 1)],
                                      in_=dram[name][128 * k:128 * (k + 1), :])
            else:
                t = const.tile(shape, dtt, tag=name, name='c_' + name)
                nc.sync.dma_start(out=t[:], in_=dram[name][:])
            cw[name] = t

        ident = cw['ident']
        vals2 = cw['vals2']
        waug = cw['w_aug']

        bf = mybir.dt.bfloat16
        decT = const.tile([27, COLS], bf, tag="decT", name="decT")
        # ones row for the logits bias (DMA: engines can't write partition 26)
        nc.sync.dma_start(out=decT[26:27, :], in_=dram['ones'][:])

        # XWC[:, t, 0:64] = XW_f step t; XWC[:, t, 64:128] = XW_b step 63-t
        XWC = const.tile([128, L, 128], dt, tag="XWC", name="XWC")

        # combined scan state: f in cols 0:64, b in cols 64:128;
        # h rows 0:13 (f) / 32:45 (b); c rows 32:45 both halves
        hcomb = const.tile([45, 128], dt, tag="hcomb", name="hcomb")
        ccomb = const.tile([45, 128], dt, tag="ccomb", name="ccomb")
        nc.vector.memset(hcomb[:], 0.0)
        nc.vector.memset(ccomb[32:45, :], 0.0)

        # ---------------- phase B head: gather + XW ----------------
        with tc.tile_pool(name="setup", bufs=1) as setup:
            embG = setup.tile([128, 32, DIN], dt, tag="embG", name="embG")
            if FLAGS['gather']:
                # split: one SWDGE ring holds 128 in-flight descriptors
                GCH = int(_os.environ.get('K_GCH', '512'))
                nch = GCH // 128
                for k in range(COLS // GCH):
                    nc.gpsimd.dma_gather(
                        embG[:, nch * k:nch * (k + 1), :], dram['emb'][:],
                        cw['xidx'][:, (GCH // 16) * k:(GCH // 16) * (k + 1)],
                        num_idxs=GCH, num_idxs_reg=GCH, elem_size=DIN)
            else:
                nc.vector.memset(embG[:], 0.01)
            embT = setup.tile([65, COLS], dt, tag="embT", name="embT")
            nc.vector.memset(embT[64:65, :], 1.0)
            for k in range(32):
                tp = psM.tile([128, 512], dt, tag="m", name=f"tpE{k}")
                nc.tensor.transpose(tp[0:DIN, 0:128], embG[:, k, :], ident[:])
                nc.scalar.copy(embT[0:DIN, 128 * k:128 * (k + 1)],
                               tp[0:DIN, 0:128])
            for ch in range(8):
                s = slice(512 * ch, 512 * (ch + 1))
                pf = psM.tile([128, 8, 64], dt, tag="m", name=f"pXWf{ch}",
                              padded_shape=[128, 8, 64])
                nc.tensor.matmul(pf[:], cw['wstk_f'][:], embT[:, s])
                nc.vector.tensor_copy(XWC[:, 8 * ch:8 * (ch + 1), 0:64], pf[:])
                pb = psM.tile([128, 8, 64], dt, tag="m", name=f"pXWb{ch}",
                              padded_shape=[128, 8, 64])
                nc.tensor.matmul(pb[:], cw['wstk_b'][:], embT[:, s])
                hi = 63 - 8 * ch
                lo = hi - 8
                rev = (slice(hi, None, -1) if lo < 0
                       else slice(hi, lo, -1))
                nc.vector.tensor_copy(XWC[:, rev, 64:128], pb[:])

        # ---------------- interleaved scan + phase A ----------------
        lt_pool = ctx.enter_context(tc.tile_pool(name="lt", bufs=2))
        enc = ctx.enter_context(tc.tile_pool(name="enc", bufs=2))
        act = ctx.enter_context(tc.tile_pool(name="act", bufs=2))

        ltiles = {}

        def emit_chunk(c):
            """decoder l>=1 cell for cols [512c, 512c+512) -> decT."""
            s = slice(512 * c, 512 * (c + 1))
            pre = psM.tile([128, 512], dt, tag="m", name=f"pre{c}")
            nc.tensor.matmul(pre[:], cw['dec_pre'][:], vals2[:, s])
            sgA = act.tile([58, 512], dt, tag="sgA", name=f"sgA{c}")
            tgA = act.tile([26, 512], dt, tag="tgA", name=f"tgA{c}")
            nc.scalar.activation(sgA[:], pre[0:58, :], AF.Sigmoid)
            nc.scalar.activation(tgA[:], pre[64:90, :], AF.Tanh)
            cD = act.tile([26, 512], dt, tag="cD", name=f"cD{c}")
            nc.vector.tensor_mul(cD[:], sgA[0:26, :], tgA[:])
            # thD lives at base 32 so the hD mul's inputs share a base
            thD = act.tile([58, 512], dt, tag="thD", name=f"thD{c}")
            nc.scalar.activation(thD[32:58, :], cD[:], AF.Tanh)
            nc.vector.tensor_mul(decT[0:26, s], sgA[32:58, :], thD[32:58, :])

        def emit_tile_half(t, half):
            """4 of 8 logits matmuls + copies for row-tile t; DMA at half 1."""
            lhs = decT[:, 128 * t:128 * (t + 1)]
            if half == 0:
                ltiles[t] = lt_pool.tile([128, V], dt, tag="ltile",
                                         name=f"ltile{t}")
            sb = ltiles[t]
            for j in range(4 * half, 4 * half + 4):
                s = slice(VCH * j, VCH * (j + 1))
                ps = psA.tile([128, VCH], dt, tag="ps", name=f"lg{t}_{j}")
                nc.tensor.matmul(ps[:], lhs, waug[:, s])
                if j < 4:
                    nc.vector.tensor_copy(sb[:, s], ps[:])
                else:
                    nc.scalar.copy(sb[:, s], ps[:])
            if half == 1:
                if t == 0:
                    nc.sync.dma_start(out=dram['out'][64:128, :],
                                      in_=sb[64:128, :])
                else:
                    nc.sync.dma_start(
                        out=dram['out'][128 * t:128 * (t + 1), :], in_=sb[:])
                del ltiles[t]

        def emit_enc_step(t):
            g = psB.tile([128, 128], dt, tag="g", name=f"g_{t}")
            nc.tensor.matmul(g[:], cw['whh_c'][:], hcomb[:])
            nc.vector.tensor_add(g[:], g[:], XWC[:, t, :])
            sg = enc.tile([77, 128], dt, tag="sg", name=f"sg_{t}")
            nc.scalar.activation(sg[:], g[0:77, :], AF.Sigmoid)
            tg = enc.tile([13, 128], dt, tag="tg", name=f"tg_{t}")
            nc.scalar.activation(tg[:], g[96:109, :], AF.Tanh)
            t1 = enc.tile([45, 128], dt, tag="t1", name=f"t1_{t}")
            nc.vector.tensor_mul(t1[32:45, :], sg[0:13, :], tg[:])
            nc.vector.tensor_mul(ccomb[32:45, :], sg[32:45, :], ccomb[32:45, :])
            nc.vector.tensor_add(ccomb[32:45, :], ccomb[32:45, :], t1[32:45, :])
            th = enc.tile([77, 128], dt, tag="th", name=f"th_{t}")
            nc.scalar.activation(th[64:77, :], ccomb[32:45, :], AF.Tanh)
            nc.vector.tensor_mul(hcomb[0:13, 0:64], sg[64:77, 0:64],
                                 th[64:77, 0:64])
            nc.vector.tensor_mul(hcomb[32:45, 64:128], sg[64:77, 64:128],
                                 th[64:77, 64:128])

        for t in range(L):
            if FLAGS['scan']:
                emit_enc_step(t)
            if FLAGS['phaseA']:
                if t % 8 == 0:
                    emit_chunk(t // 8)
                emit_tile_half(t // 2, t % 2)

        # ---------------- phase B tail ----------------
        if not FLAGS['tail']:
            dummy = const.tile([1, 16], dt, tag="dummy", name="dummy")
            nc.vector.memset(dummy[:], 0.0)
            nc.sync.dma_start(out=dram['out'][0:1, 0:16], in_=dummy[:])
            return
        # assemble padded [hf@0:13, hb@32:45, ones@64] rhs tiles
        hboth = const.tile([128, NLOC], dt, tag="hboth", name="hboth")
        cboth = const.tile([128, NLOC], dt, tag="cboth", name="cboth")
        nc.vector.memset(hboth[:], 0.0)
        nc.vector.tensor_copy(hboth[0:13, :], hcomb[0:13, 0:64])
        nc.vector.tensor_copy(hboth[32:45, :], hcomb[32:45, 64:128])
        nc.vector.memset(hboth[64:65, :], 1.0)
        nc.vector.memset(cboth[:], 0.0)
        nc.vector.tensor_copy(cboth[0:13, :], ccomb[32:45, 0:64])
        nc.vector.tensor_copy(cboth[32:45, :], ccomb[32:45, 64:128])
        nc.vector.memset(cboth[64:65, :], 1.0)
        pst = psM.tile([128, 512], dt, tag="m", name="pst")
        nc.tensor.matmul(pst[0:26, 0:NLOC], cw['proj1'][:], hboth[:],
                         skip_group_check=True)
        nc.tensor.matmul(pst[32:58, 0:NLOC], cw['proj2'][:], cboth[:],
                         skip_group_check=True)
        state = const.tile([64, NLOC], dt, tag="state", name="state")
        nc.vector.memset(state[:], 0.0)
        nc.vector.tensor_copy(state[0:26, :], pst[0:26, 0:NLOC])
        nc.vector.tensor_copy(state[32:58, :], pst[32:58, 0:NLOC])
        pstN = psM.tile([128, 512], dt, tag="m", name="pstN")
        nc.tensor.transpose(pstN[0:NLOC, 0:64], state[:], ident[0:64, 0:64])
        stateN = const.tile([NLOC, 64], dt, tag="stateN", name="stateN")
        nc.vector.tensor_copy(stateN[:], pstN[0:NLOC, 0:64])
        nc.sync.dma_start(out=dram['cc_in'][:], in_=stateN[:])
        if FLAGS['collective']:
            nc.gpsimd.collective_compute(
                "AllGather", mybir.AluOpType.bypass,
                ins=[dram['cc_in'][:]], outs=[dram['cc_out'][:]],
                replica_groups=[list(range(NC))])
        else:
            for _r in range(NC):
                nc.sync.dma_start(out=dram['cc_out'][_r * NLOC:(_r + 1) * NLOC, :],
                                  in_=dram['cc_in'][:])
        # read back + transpose to [26, 512] x2 (h rows 0:26, c rows 32:58)
        XhT = const.tile([26, N], dt, tag="XhT", name="XhT")
        XcT = const.tile([26, N], dt, tag="XcT", name="XcT")
        for k in range(4):
            xn = work.tile([128, 64], dt, tag="xn", name=f"xn{k}")
            nc.sync.dma_start(out=xn[:],
                              in_=dram['cc_out'][128 * k:128 * (k + 1), :])
            tp = psM.tile([128, 512], dt, tag="m", name=f"tpX{k}")
            nc.tensor.transpose(tp[0:64, 0:128], xn[:], ident[:])
            nc.vector.tensor_copy(XhT[:, 128 * k:128 * (k + 1)],
                                  tp[0:26, 0:128])
            nc.vector.tensor_copy(XcT[:, 128 * k:128 * (k + 1)],
                                  tp[32:58, 0:128])

        # GCN x2 (gh on XhT, gc on XcT) -> padded [46, 64] local outputs
        outT = {}
        for g, XT in (('gh', XhT), ('gc', XcT)):
            m1 = work.tile([128, 64], dt, tag="m1", name=f"m1{g}")
            for k in range(4):
                pm = psM.tile([128, 512], dt, tag="m", name=f"pm1{g}{k}")
                nc.tensor.matmul(pm[0:128, 0:16],
                                 XT[:, 128 * k:128 * (k + 1)], cw[f'{g}_W1'][:])
                nc.vector.tensor_copy(m1[:, 16 * k:16 * (k + 1)],
                                      pm[0:128, 0:16])
            agg1 = psM.tile([128, 512], dt, tag="m", name=f"agg1{g}")
            for k in range(4):
                nc.tensor.matmul(agg1[0:16, :], m1[:, 16 * k:16 * (k + 1)],
                                 cw['adjT'][:, 512 * k:512 * (k + 1)],
                                 start=(k == 0), stop=(k == 3))
            xn1 = work.tile([16, N], dt, tag="xn1", name=f"xn1{g}")
            nc.scalar.activation(xn1[:], agg1[0:16, :], AF.Lrelu,
                                 bias=cw[f'{g}_b1'][:], alpha=0.01)
            m2 = work.tile([128, 128], dt, tag="m2", name=f"m2{g}")
            for k in range(4):
                pm = psM.tile([128, 512], dt, tag="m", name=f"pm2{g}{k}")
                nc.tensor.matmul(pm[0:128, 0:32],
                                 xn1[:, 128 * k:128 * (k + 1)], cw[f'{g}_W2'][:])
                nc.vector.tensor_copy(m2[:, 32 * k:32 * (k + 1)],
                                      pm[0:128, 0:32])
            agg2 = psM.tile([128, 512], dt, tag="m", name=f"agg2{g}")
            for k in range(4):
                nc.tensor.matmul(agg2[0:32, 0:NLOC], m2[:, 32 * k:32 * (k + 1)],
                                 cw['adjT_loc'][:, NLOC * k:NLOC * (k + 1)],
                                 start=(k == 0), stop=(k == 3))
            xn2 = work.tile([32, NLOC], dt, tag="xn2", name=f"xn2{g}")
            nc.scalar.activation(xn2[:], agg2[0:32, 0:NLOC], AF.Lrelu,
                                 bias=cw[f'{g}_b2'][:], alpha=0.01)
            pf = psM.tile([128, 512], dt, tag="m", name=f"pf{g}")
            nc.tensor.matmul(pf[0:46, 0:NLOC], cw[f'{g}_Wfc'][:], xn2[:])
            oT = work.tile([46, NLOC], dt, tag=f"oT{g}", name=f"oT{g}")
            nc.scalar.activation(oT[:], pf[0:46, 0:NLOC], AF.Identity,
                                 bias=cw[f'{g}_bfc'][:])
            outT[g] = oT
        shT, scT = outT['gh'], outT['gc']

        # decoder step 0 -> dec0_pad [hF@0:13, hB@32:45, ones@96]
        dec0p = const.tile([128, NLOC], dt, tag="dec0p", name="dec0p")
        nc.vector.memset(dec0p[:], 0.0)
        nc.vector.memset(dec0p[96:97, :], 1.0)
        for di, wn in ((0, 'dec0_f'), (1, 'dec0_b')):
            h0 = work.tile([64, NLOC], dt, tag=f"h0{di}", name=f"h0{di}")
            nc.vector.memset(h0[:], 0.0)
            nc.vector.tensor_copy(h0[0:13, :], shT[32 * di:32 * di + 13, :])
            nc.vector.memset(h0[32:33, :], 1.0)
            g0 = psM.tile([128, 512], dt, tag="m", name=f"g0{di}")
            nc.tensor.matmul(g0[:, 0:NLOC], cw[wn][:], h0[:])
            sg = work.tile([77, NLOC], dt, tag=f"d0sg{di}", name=f"d0sg{di}")
            nc.scalar.activation(sg[:], g0[0:77, 0:NLOC], AF.Sigmoid)
            tg = work.tile([13, NLOC], dt, tag=f"d0tg{di}", name=f"d0tg{di}")
            nc.scalar.activation(tg[:], g0[96:109, 0:NLOC], AF.Tanh)
            sc0 = work.tile([45, NLOC], dt, tag=f"d0sc{di}", name=f"d0sc{di}")
            nc.vector.tensor_copy(sc0[32:45, :], scT[32 * di:32 * di + 13, :])
            c0 = work.tile([45, NLOC], dt, tag=f"d0c{di}", name=f"d0c{di}")
            nc.vector.tensor_mul(c0[32:45, :], sg[0:13, :], tg[:])
            ct = work.tile([45, NLOC], dt, tag=f"d0ct{di}", name=f"d0ct{di}")
            nc.vector.tensor_mul(ct[32:45, :], sg[32:45, :], sc0[32:45, :])
            nc.vector.tensor_add(c0[32:45, :], c0[32:45, :], ct[32:45, :])
            th = work.tile([77, NLOC], dt, tag=f"d0th{di}", name=f"d0th{di}")
            nc.scalar.activation(th[64:77, :], c0[32:45, :], AF.Tanh)
            nc.vector.tensor_mul(dec0p[32 * di:32 * di + 13, :],
                                 sg[64:77, :], th[64:77, :])

        # pack to [27, 64] and emit l=0 logits -> out rows 0:64
        ppk = psM.tile([128, 512], dt, tag="m", name="ppk")
        nc.tensor.matmul(ppk[0:27, 0:NLOC], cw['packP27'][:], dec0p[:])
        dec27 = const.tile([27, NLOC], bf, tag="dec27", name="dec27")
        nc.vector.tensor_copy(dec27[:], ppk[0:27, 0:NLOC])
        l0 = const.tile([NLOC, V], dt, tag="l0", name="l0")
        for j in range(8):
            s = slice(VCH * j, VCH * (j + 1))
            ps = psA.tile([128, VCH], dt, tag="ps", name=f"l0g{j}")
            nc.tensor.matmul(ps[0:NLOC, :], dec27[:], waug[:, s])
            nc.vector.tensor_copy(l0[:, s], ps[0:NLOC, :])
        nc.sync.dma_start(out=dram['out'][0:NLOC, :], in_=l0[:])


# ---------------------------------------------------------------------------
# entry point
# ---------------------------------------------------------------------------
def kernel(x=None, edge_index=None, params=None, **kw):
    from concourse.bass_utils import run_bass_kernel_spmd
    S, P = _make_host_inputs(x, edge_index, params)
    nc = _build()

    import ml_dtypes
    per_core_keys = {'xidx', 'vals2', 'adjT_loc'}
    conv = {}
    for name, _, kind in INPUT_SPECS:
        if kind == 'bf16' and name not in per_core_keys:
            conv[name] = S[name].astype(ml_dtypes.bfloat16)
    in_maps = []
    for c in range(NC):
        m = {}
        for name, _, kind in INPUT_SPECS:
            if name in per_core_keys:
                m[name] = P[c][name]
            else:
                m[name] = conv.get(name, S[name])
        in_maps.append(m)

    res = run_bass_kernel_spmd(nc, in_maps, core_ids=list(range(NC)), **kw)
    out = np.empty((N, L, V), np.float32)
    for c in range(NC):
        shard = res.results[c]['out'].reshape(L, NLOC, V)
        out[c * NLOC:(c + 1) * NLOC] = shard.transpose(1, 0, 2)
    if kw.get('trace'):
        kernel.last_exec_time_ns = res.exec_time_ns
    return out
